# revision 19
# baseline (speedup 1.0000x reference)
"""BatchedLightSimulation Trainium2 kernel.

Math: the two causal convolutions (scintillation 990 taps, SiPM impulse 990
taps) compose into one 1979-tap causal filter c.  Folding the sum-by-16
downsample in gives

    out[row, s] = gain[row] * sum_delta c16[delta] * u[row, 16*s + delta]

with c16[delta] = sum_{k=max(0,delta)}^{15} c[k - delta].  c decays like
exp(-l/15.3) so c16 truncated to delta >= -240 is exact at fp32 precision
(validated 4e-7 of absmax vs the jax reference).

Device mapping (per core, 4 ninputs = 192 (n,d) rows):
  polyphase m = 16q + r.  SBUF tile X[q, st, r, h, row] holds, for each of
  4 output s-tiles (100 s each) and 16 phases r, the hi/lo bf16 split of
  u[row, 16*(s0-15+q) + r].  Per (st, r) two matmuls accumulate into
  psum[128, 384]:
      whi.T @ [xhi | xlo]   -> cols [0:192] += hi*hi, [192:384] += hi*lo
      wlo.T @ xhi           -> cols [0:192] += lo*hi
  (u = uhi+ulo and W = whi+wlo are exact bf16 splits to ~2^-17; bf16
  products accumulate exactly in fp32 PSUM; the dropped lo*lo term is
  ~2^-18.  End-to-end error vs the reference: 2.9e-6 of absmax; a pure
  fp32 path measures 5e-7 but runs 2.7x slower on the PE because fp32
  matmuls self-load weights in 2 half-passes with no background weight
  buffer.)  Epilogue per s-tile: DVE adds the psum halves, PE transposes
  to [row, s], DVE applies the per-row gain during the PSUM->SBUF copy,
  and the s-tile slice is DMA'd out.

Perf notes (all measured on TRN2 via NTFF profiles):
  - The host ships each core's shard already split and in the polyphase
    layout (a pure permutation + hi/lo dtype split done during the
    mandatory shard-and-copy step) so the input DMA is fully contiguous:
    a 64B-chunk gather DMA measures ~38 GB/s vs ~300 GB/s contiguous.
  - SBUF tiles with 115 partitions DMA 3.3x slower than 128-partition
    tiles, so the q-window (115 live rows) is padded to 128.
  - Weight columns are padded to 128 so bf16 fast-weight-load kicks in;
    rhs rows are innermost so the moving operand streams stride-1
    (strided rhs slows the matmul ~3x).
  - x transfers are split per half-s-tile across both HWDGE rings in
    consumption order; ~40 dummy matmuls on a memset tile bridge the HAM
    clock-gate warmup (1.2 -> 2.4 GHz) while the first chunks land.
"""

import numpy as np
import ml_dtypes

import concourse.bacc as bacc
import concourse.mybir as mybir
import concourse.tile as tile
from concourse.bass_utils import run_bass_kernel_spmd

# ---- problem constants (hardcoded per contract) ----
NINPUT, NDET, NTICK = 32, 48, 6400
NS = 16                    # downsample factor
S = NTICK // NS            # 400 output ticks
LIGHT_TICK = 0.1
CONV_TICKS = 990
NCORES = 8
N_PER_CORE = NINPUT // NCORES      # 4
ROWS = N_PER_CORE * NDET           # 192 rows per core
J = 15                             # q-steps of history (taps delta >= -16*J)
HALO = J
PAD = NS * HALO                    # 240 zero ticks prepended
TPAD = NTICK + PAD                 # 6640
STILE = 100                        # s-values per output tile
NST = S // STILE                   # 4
QW = STILE + HALO                  # 115 live q rows per tile
QP = 128                           # padded q partitions (DMA speed)
DMAX = NS * J                      # 240
N_WARM = 44                        # dummy matmuls to lift the HAM clock gate
WCOL = 128                         # weight columns (128 enables FWL)
XFREE = NST * ROWS * NS            # 12288
TALLOC = NS * STILE * (NST - 1) + NS * QP + NS   # 6848: strided-view extent

BF16 = ml_dtypes.bfloat16


def _build_taps(singlet_fraction_logit, log_tau_s, log_tau_t,
                light_oscillation_period, light_response_time):
    """c16[delta] for delta in [-DMAX, 15], float64."""
    dt = float(LIGHT_TICK)
    tt = np.arange(CONV_TICKS, dtype=np.float64)
    sf = 1.0 / (1.0 + np.exp(-float(singlet_fraction_logit)))
    tau_s = 10.0 ** float(log_tau_s)
    tau_t = 10.0 ** float(log_tau_t)
    per = float(light_oscillation_period)
    rt = float(light_response_time)
    p1 = sf * np.exp(-tt * dt / tau_s) * (1.0 - np.exp(-dt / tau_s))
    p3 = (1.0 - sf) * np.exp(-tt * dt / tau_t) * (1.0 - np.exp(-dt / tau_t))
    scint = p1 + p3
    t = tt * dt
    imp = np.exp(-t / rt) * np.sin(t / per)
    imp = imp / (per * rt * rt) * (per * per + rt * rt) * dt
    c = np.convolve(scint, imp)          # length 2*990-1 = 1979
    deltas = np.arange(-DMAX, 16)
    c16 = np.zeros(len(deltas), dtype=np.float64)
    for i, d in enumerate(deltas):
        ks = np.arange(max(0, d), 16)
        c16[i] = c[ks - d].sum()
    return c16                            # index i -> delta = i - DMAX


def _build_weights(c16):
    """W[q_rel, r, s_rel] float32 (QP rows, WCOL cols, zero-padded)."""
    w = np.zeros((QP, NS, WCOL), dtype=np.float64)
    q_rel = np.arange(QP)[:, None, None]
    r = np.arange(NS)[None, :, None]
    s_rel = np.arange(WCOL)[None, None, :]
    delta = 16 * (q_rel - HALO - s_rel) + r
    mask = ((delta >= -DMAX) & (delta <= 15) & (q_rel < QW)
            & (s_rel < STILE))
    w[mask] = c16[(delta + DMAX)[mask]]
    return np.ascontiguousarray(w, dtype=np.float32)


def _split_bf16(a):
    hi = a.astype(BF16)
    lo = (a - hi.astype(np.float32)).astype(BF16)
    return hi, lo


_PROGRAM = None


def _build_program():
    global _PROGRAM
    if _PROGRAM is not None:
        return _PROGRAM
    nc = bacc.Bacc("TRN2", target_bir_lowering=False, debug=False,
                   enable_asserts=False, num_devices=NCORES)
    f32 = mybir.dt.float32
    bf16 = mybir.dt.bfloat16
    x_d = nc.dram_tensor("x", [QP, 2 * XFREE], bf16, kind="ExternalInput")
    wh_d = nc.dram_tensor("whi", [QP, NS * WCOL], bf16, kind="ExternalInput")
    wl_d = nc.dram_tensor("wlo", [QP, NS * WCOL], bf16, kind="ExternalInput")
    g_d = nc.dram_tensor("gain", [128, 2], f32, kind="ExternalInput")
    i_d = nc.dram_tensor("ident", [128, 128], f32, kind="ExternalInput")
    o_d = nc.dram_tensor("out", [ROWS, S], f32, kind="ExternalOutput")

    XQ = 2 * XFREE // NST    # one s-tile's x extent (hi+lo)

    with tile.TileContext(nc) as tc:
        with (
            tc.tile_pool(name="const", bufs=1) as cpool,
            tc.tile_pool(name="x", bufs=1) as xpool,
            tc.tile_pool(name="ep", bufs=2) as epool,
            tc.tile_pool(name="fin", bufs=1) as fpool,
            tc.tile_pool(name="ps", bufs=1, space="PSUM") as pspool,
            tc.tile_pool(name="warm", bufs=1, space="PSUM") as wpool,
            tc.tile_pool(name="psT", bufs=1, space="PSUM") as ptpool,
        ):
            # PE warm-up: dummy bf16 matmuls on a memset tile (no DMA
            # dependency) keep TensorE busy from ~2us so the HAM clock
            # gate opens (1.2 -> 2.4 GHz) before the real matmuls start.
            warm_w = cpool.tile([128, 256], bf16, tag="warmw")
            nc.vector.memset(warm_w[:], 1.0)
            ps_warm = wpool.tile([128, 256], f32, tag="warm")
            for _ in range(N_WARM):
                nc.tensor.matmul(ps_warm[:], warm_w[:, 0:128], warm_w[:],
                                 start=True, stop=True)

            # one weight tensor first on each ring; the first matmuls are
            # gated on whi/wlo + the first x chunk only.
            wh_sb = cpool.tile([QP, NS * WCOL], bf16, tag="wh")
            nc.sync.dma_start(wh_sb[:], wh_d[:])
            wl_sb = cpool.tile([QP, NS * WCOL], bf16, tag="wl")
            nc.scalar.dma_start(wl_sb[:], wl_d[:])

            # x[q, st, r, h, row]: h in {hi, lo}; row contiguous so the
            # matmul moving operand streams stride-1.  Half s-tile per DMA
            # (8 r-phases); each s-tile's halves go to different rings and
            # chunks are issued in consumption order.
            x_sb = xpool.tile([QP, NST, NS, 2, ROWS], bf16, tag="x")
            x_flat = x_sb[:].rearrange("q st r h row -> q (st r h row)")
            g_sb = cpool.tile([128, 2], f32, tag="g")
            id_sb = cpool.tile([128, 128], f32, tag="id")
            for st in range(NST):
                for hh in range(2):
                    eng = nc.sync if hh == 0 else nc.scalar
                    lo = st * XQ + hh * XQ // 2
                    eng.dma_start(x_flat[:, lo:lo + XQ // 2],
                                  x_d[:, lo:lo + XQ // 2])
                if st == 0:
                    nc.scalar.dma_start(g_sb[:], g_d[:])
                    nc.scalar.dma_start(id_sb[:], i_d[:])

            fin_a = fpool.tile([128, S], f32, tag="fa")
            fin_b = fpool.tile([64, S], f32, tag="fb")

            # all matmuls first (the PE-critical path); epilogues after,
            # so Tile slots the transposes into PE gaps instead of
            # stalling the matmul stream at s-tile boundaries.
            ps_tiles = []
            for st in range(NST):
                ps = pspool.tile([WCOL, 2 * ROWS], f32, tag=f"ps{st}")
                ps_tiles.append(ps)
                # all whi matmuls first: they are gated on whi + x only,
                # so the stream is not stalled waiting for wlo's DMA
                for r in range(NS):
                    wh = wh_sb[:, r * WCOL:(r + 1) * WCOL]
                    nc.tensor.matmul(
                        ps[:], wh, x_sb[:, st, r, :, :],
                        start=(r == 0), stop=False,
                    )
                for r in range(NS):
                    wl = wl_sb[:, r * WCOL:(r + 1) * WCOL]
                    nc.tensor.matmul(
                        ps[:, 0:ROWS], wl, x_sb[:, st, r, 0, :],
                        start=False, stop=(r == NS - 1),
                    )
            for st in range(NST):
                ps = ps_tiles[st]
                sl = slice(st * STILE, (st + 1) * STILE)
                t_lo = epool.tile([STILE, ROWS], f32, tag="tlo")
                nc.vector.tensor_copy(t_lo[:], ps[0:STILE, ROWS:2 * ROWS])
                summed = epool.tile([STILE, ROWS], f32, tag="summed")
                nc.vector.tensor_add(summed[:], ps[0:STILE, 0:ROWS], t_lo[:])
                pT_a = ptpool.tile([128, STILE], f32, tag="pTa")
                nc.tensor.transpose(pT_a[:], summed[:, 0:128],
                                    id_sb[0:STILE, 0:STILE])
                pT_b = ptpool.tile([64, STILE], f32, tag="pTb")
                nc.tensor.transpose(pT_b[:], summed[:, 128:ROWS],
                                    id_sb[0:STILE, 0:STILE])
                # gain is per-row = per-partition after the transpose:
                # fold it into the PSUM->SBUF copy as a tensor_scalar
                nc.vector.tensor_scalar_mul(fin_a[:, sl], pT_a[:],
                                            g_sb[:, 0:1])
                nc.vector.tensor_scalar_mul(fin_b[:, sl], pT_b[:],
                                            g_sb[0:64, 1:2])

            nc.scalar.dma_start(o_d[0:128, 0:3 * STILE],
                                fin_a[:, 0:3 * STILE])
            nc.scalar.dma_start(o_d[128:ROWS, 0:3 * STILE],
                                fin_b[:, 0:3 * STILE])
            nc.scalar.dma_start(o_d[0:128, 3 * STILE:S],
                                fin_a[:, 3 * STILE:S])
            nc.scalar.dma_start(o_d[128:ROWS, 3 * STILE:S],
                                fin_b[:, 3 * STILE:S])

    nc.compile()
    _PROGRAM = nc
    return nc


def _prepare_inputs(timing_dist, singlet_fraction_logit, log_tau_s, log_tau_t,
                    light_oscillation_period, light_response_time, light_gain):
    u = np.ascontiguousarray(np.asarray(timing_dist, dtype=np.float32))
    assert u.shape == (NINPUT, NDET, NTICK)
    gain = np.asarray(light_gain, dtype=np.float32).reshape(NDET)

    c16 = _build_taps(singlet_fraction_logit, log_tau_s, log_tau_t,
                      light_oscillation_period, light_response_time)
    w = _build_weights(c16).reshape(QP, NS * WCOL)
    whi, wlo = _split_bf16(w)

    gain_row = np.tile(gain, N_PER_CORE)                     # [ROWS]
    gain_col = np.zeros((128, 2), dtype=np.float32)
    gain_col[:, 0] = gain_row[0:128]
    gain_col[0:64, 1] = gain_row[128:192]
    ident = np.eye(128, dtype=np.float32)

    in_maps = []
    for c in range(NCORES):
        shard = u[c * N_PER_CORE:(c + 1) * N_PER_CORE].reshape(ROWS, NTICK)
        up = np.zeros((ROWS, TALLOC), dtype=np.float32)
        up[:, PAD:TPAD] = shard
        # polyphase relayout: x[q, st, r, h, row] = split_h(
        #     up[row, 1600*st + 16*q + r])
        uphi, uplo = _split_bf16(up)
        xs = []
        for a in (uphi, uplo):
            xs.append(np.lib.stride_tricks.as_strided(
                a,
                shape=(QP, NST, NS, ROWS),
                strides=(NS * 2, NS * STILE * 2, 2, a.strides[0]),
            ))
        x = np.ascontiguousarray(np.stack(xs, axis=3)).reshape(QP, 2 * XFREE)
        in_maps.append({"x": x, "whi": whi, "wlo": wlo,
                        "gain": gain_col, "ident": ident})
    return in_maps


def _run(in_maps, trace=False):
    nc = _build_program()
    res = run_bass_kernel_spmd(nc, in_maps, core_ids=list(range(NCORES)),
                               trace=trace)
    outs = [res.results[c]["out"].reshape(N_PER_CORE, NDET, S)
            for c in range(NCORES)]
    full = np.concatenate(outs, axis=0).astype(np.float32, copy=False)
    return full, res


def kernel(timing_dist, singlet_fraction_logit, log_tau_s, log_tau_t,
           light_oscillation_period, light_response_time, light_gain):
    in_maps = _prepare_inputs(
        timing_dist, singlet_fraction_logit, log_tau_s, log_tau_t,
        light_oscillation_period, light_response_time, light_gain)
    full, _ = _run(in_maps, trace=False)
    return full
